# revision 13
# baseline (speedup 1.0000x reference)
"""CrossCompressUnit kernel for Trainium2 (8 NeuronCores, SPMD data-parallel).

Computes, for row-batches v, e of shape (B, 64):
    a = e @ w_vv ; b = v @ w_ev ; c = e @ w_ve ; d = v @ w_ee   (per-row dots)
    v_out = v * a + e * b + bias_v
    e_out = v * c + e * d + bias_e

Sharding: pure data parallel over the batch axis across 8 cores; the tiny
(64,1) weights / (64,) biases are replicated to every core.

Layout per core (B_c = 32768 rows):
    DRAM [32768, 64] viewed as [128 partitions, 256 rows, 64]; tiles of
    R rows/partition are processed with 3D access patterns. Per-row dot
    products use a tensor_tensor multiply against a partition-replicated
    weight tile followed by a grouped (axis=X) reduce; the per-row scalars
    are then broadcast back along the dim axis with stride-0 access
    patterns.
"""

import os
import sys

import numpy as np

for _p in (
    "/root/.axon_site",
    "/root/.axon_site/_ro/trn_rl_repo",
    "/root/.axon_site/_ro/pypackages",
    "/opt/trn_rl_repo",
):
    if os.path.isdir(_p) and _p not in sys.path:
        sys.path.append(_p)

import concourse.bacc as bacc
import concourse.bass as bass
import concourse.mybir as mybir
import concourse.tile as tile
from concourse.bass_utils import run_bass_kernel_spmd

BATCH = 262144
DIM = 64
NCORES = 8
BC = BATCH // NCORES          # 32768 rows per core
P = 128                       # SBUF partitions
QROWS = BC // P               # 256 rows per partition
R = 32                        # rows per partition per tile
NT = QROWS // R               # tiles per core

F32 = mybir.dt.float32
MUL = mybir.AluOpType.mult
ADD = mybir.AluOpType.add
AX = mybir.AxisListType.X

_cache = {}


def _build_program(with_bias: bool) -> bass.Bass:
    # Bacc (not raw Bass): its compile() pass pipeline splits semaphore
    # waits (generate_event_semaphores) to satisfy the TRN2 1-wait-per-
    # instruction constraint that raw Tile output violates.
    nc = bacc.Bacc("TRN2", target_bir_lowering=False, debug=False)

    v_d = nc.dram_tensor("v", [BC, DIM], F32, kind="ExternalInput").ap()
    e_d = nc.dram_tensor("e", [BC, DIM], F32, kind="ExternalInput").ap()
    w_d = nc.dram_tensor("wrep", [P, 4, DIM], F32, kind="ExternalInput").ap()
    if with_bias:
        b_d = nc.dram_tensor("brep", [P, 2, DIM], F32, kind="ExternalInput").ap()
    vo_d = nc.dram_tensor("v_out", [BC, DIM], F32, kind="ExternalOutput").ap()
    eo_d = nc.dram_tensor("e_out", [BC, DIM], F32, kind="ExternalOutput").ap()

    vv = v_d.rearrange("(p q) d -> p q d", p=P)
    ee = e_d.rearrange("(p q) d -> p q d", p=P)
    vvo = vo_d.rearrange("(p q) d -> p q d", p=P)
    eeo = eo_d.rearrange("(p q) d -> p q d", p=P)

    with tile.TileContext(nc) as tc:
        with (
            tc.tile_pool(name="io", bufs=2) as io_pool,
            tc.tile_pool(name="scratch", bufs=2) as scratch,
            tc.tile_pool(name="dotp", bufs=2) as dotp,
            tc.tile_pool(name="const", bufs=1) as const_pool,
        ):
            # SWDGE (single completion semaphore) for the tiny constant
            # tensors: consumers already wait on the multi-queue HWDGE sems
            # of the big input DMAs, and the TT encoding has few wait slots.
            wt = const_pool.tile([P, 4, DIM], F32, tag="w")
            nc.gpsimd.dma_start(wt[:], w_d)
            if with_bias:
                bt = const_pool.tile([P, 2, DIM], F32, tag="b")
                nc.gpsimd.dma_start(bt[:], b_d)

            def wb(k):  # weight k broadcast over R rows: [P, R, DIM]
                return wt[:, k, :].unsqueeze(1).broadcast_to([P, R, DIM])

            def sb(dots_ap):  # per-row scalar broadcast along dim
                return dots_ap.unsqueeze(-1).broadcast_to([P, R, DIM])

            for j in range(NT):
                sl = slice(j * R, (j + 1) * R)
                # SWDGE loads: one completion semaphore per DMA, so the
                # first consuming TT stays within its sync-wait slot budget
                # (HWDGE sprays several queues, each with its own sem).
                vt = io_pool.tile([P, R, DIM], F32, tag="vin")
                nc.gpsimd.dma_start(vt[:], vv[:, sl, :])
                et = io_pool.tile([P, R, DIM], F32, tag="ein")
                nc.gpsimd.dma_start(et[:], ee[:, sl, :])

                dots = dotp.tile([P, 4, R], F32, tag="dots")
                # a = e.w_vv, b = v.w_ev, c = e.w_ve, d = v.w_ee
                for k, src in ((0, et), (1, vt), (2, et), (3, vt)):
                    prod = scratch.tile([P, R, DIM], F32, tag=f"prod{k}")
                    nc.vector.tensor_tensor(prod[:], src[:], wb(k), MUL)
                    nc.vector.tensor_reduce(dots[:, k, :], prod[:], axis=AX, op=ADD)

                for (ka, kb, out_ap, bias_k) in ((0, 1, vvo, 0), (2, 3, eeo, 1)):
                    t1 = scratch.tile([P, R, DIM], F32, tag="t1")
                    nc.vector.tensor_tensor(t1[:], vt[:], sb(dots[:, ka, :]), MUL)
                    t2 = scratch.tile([P, R, DIM], F32, tag="t2")
                    nc.vector.tensor_tensor(t2[:], et[:], sb(dots[:, kb, :]), MUL)
                    ot = io_pool.tile([P, R, DIM], F32, tag=f"out{bias_k}")
                    nc.vector.tensor_tensor(ot[:], t1[:], t2[:], ADD)
                    if with_bias:
                        bb = bt[:, bias_k, :].unsqueeze(1).broadcast_to([P, R, DIM])
                        nc.vector.tensor_tensor(ot[:], ot[:], bb, ADD)
                    nc.sync.dma_start(out_ap[:, sl, :], ot[:])
    nc.compile()
    return nc


def _build_program_c(with_bias: bool) -> bass.Bass:
    """PE-based dots: 2-row-packed transposes + block-diag weight matmuls.

    Per 2-row chunk c of a tile, transpose [128, 2, 64] -> [128, 128] PSUM
    (partitions = (row-parity, dim), columns = original partitions). A
    matmul with a block-diagonal [128, 4] lhsT then yields both rows' two
    dot products per column. Dots are transposed back to row-major [128, 8]
    j-layout [b0,d0,b1,d1,a0,c0,a1,c1] per chunk for the DVE combine.
    """
    nc = bacc.Bacc("TRN2", target_bir_lowering=False, debug=False)

    v_d = nc.dram_tensor("v", [BC, DIM], F32, kind="ExternalInput").ap()
    e_d = nc.dram_tensor("e", [BC, DIM], F32, kind="ExternalInput").ap()
    # wblob: [:, 0:4] lhsT_v (b,d block-diag), [:, 4:8] lhsT_e (a,c),
    # [:, 8:136] f32 identity
    w_d = nc.dram_tensor("wblob", [P, 136], F32, kind="ExternalInput").ap()
    if with_bias:
        b_d = nc.dram_tensor("brep", [P, 2, DIM], F32, kind="ExternalInput").ap()
    vo_d = nc.dram_tensor("v_out", [BC, DIM], F32, kind="ExternalOutput").ap()
    eo_d = nc.dram_tensor("e_out", [BC, DIM], F32, kind="ExternalOutput").ap()

    vv = v_d.rearrange("(p q) d -> p q d", p=P)
    ee = e_d.rearrange("(p q) d -> p q d", p=P)
    vvo = vo_d.rearrange("(p q) d -> p q d", p=P)
    eeo = eo_d.rearrange("(p q) d -> p q d", p=P)

    NCHUNK = R // 2            # 2-row chunks per tile
    GSZ = 4                    # chunks per PSUM staging group
    NGRP = NCHUNK // GSZ

    with tile.TileContext(nc) as tc:
        with (
            tc.tile_pool(name="io", bufs=2) as io_pool,
            tc.tile_pool(name="xsb", bufs=2) as xsb_pool,
            tc.tile_pool(name="dsb", bufs=2) as dsb_pool,
            tc.tile_pool(name="scrd", bufs=1) as scr,
            tc.tile_pool(name="scrp", bufs=2) as scrp,
            tc.tile_pool(name="const", bufs=1) as const_pool,
            tc.tile_pool(name="ps", bufs=2, space="PSUM") as ps_pool,
        ):
            wt = const_pool.tile([P, 136], F32, tag="w")
            nc.gpsimd.dma_start(wt[:], w_d)
            ident = wt[:, 8:136]
            if with_bias:
                bt = const_pool.tile([P, 2, DIM], F32, tag="b")
                nc.gpsimd.dma_start(bt[:], b_d)

            for j in range(NT):
                sl = slice(j * R, (j + 1) * R)
                vt = io_pool.tile([P, R, DIM], F32, tag="vin")
                nc.gpsimd.dma_start(vt[:], vv[:, sl, :])
                et = io_pool.tile([P, R, DIM], F32, tag="ein")
                nc.gpsimd.dma_start(et[:], ee[:, sl, :])

                xvsb = xsb_pool.tile([P, NCHUNK * P], F32, tag="xv")
                xesb = xsb_pool.tile([P, NCHUNK * P], F32, tag="xe")
                dots_sb = dsb_pool.tile([8, NCHUNK * P], F32, tag="dsb")
                dots_t_ps = ps_pool.tile([P, NCHUNK * 8], F32, tag="dtp")

                for g in range(NGRP):
                    for (src, xsb, w_lo) in ((vt, xvsb, 0), (et, xesb, 4)):
                        xstage = ps_pool.tile([P, GSZ * P], F32, tag="xstage")
                        for i in range(GSZ):
                            c = g * GSZ + i
                            nc.tensor.transpose(
                                xstage[:, i * P : (i + 1) * P],
                                src[:, 2 * c : 2 * c + 2, :],
                                ident,
                            )
                        nc.scalar.copy(
                            xsb[:, g * GSZ * P : (g + 1) * GSZ * P], xstage[:]
                        )
                    # separate PSUM tiles: matmul out base partition must be
                    # 0/32/64 (PE quadrant rule); merge into [8, N] in SBUF
                    dots_pv = ps_pool.tile([4, GSZ * P], F32, tag="dpv")
                    dots_pe = ps_pool.tile([4, GSZ * P], F32, tag="dpe")
                    gs = slice(g * GSZ * P, (g + 1) * GSZ * P)
                    nc.tensor.matmul(
                        dots_pv[:], wt[:, 0:4], xvsb[:, gs], start=True, stop=True
                    )
                    nc.tensor.matmul(
                        dots_pe[:], wt[:, 4:8], xesb[:, gs], start=True, stop=True
                    )
                    nc.scalar.copy(dots_sb[0:4, gs], dots_pv[:])
                    nc.scalar.copy(dots_sb[4:8, gs], dots_pe[:])
                    for i in range(GSZ):
                        c = g * GSZ + i
                        nc.tensor.transpose(
                            dots_t_ps[:, c * 8 : (c + 1) * 8],
                            dots_sb[:, c * P : (c + 1) * P],
                            ident[0:8, 0:8],
                        )

                dots_t = dsb_pool.tile([P, NCHUNK * 8], F32, tag="dt")
                nc.vector.tensor_copy(dots_t[:], dots_t_ps[:])
                dview = dots_t[:].rearrange("p (c j) -> p c j", j=8)

                def dot_bc(off):
                    return (
                        dview[:, :, off : off + 3 : 2]
                        .unsqueeze(-1)
                        .broadcast_to([P, NCHUNK, 2, DIM])
                    )

                v4 = vt[:].rearrange("p (c t) d -> p c t d", t=2)
                e4 = et[:].rearrange("p (c t) d -> p c t d", t=2)

                # v_out = a*v + b*e ; e_out = c*v + d*e
                # j-layout per chunk: [b0, d0, b1, d1, a0, c0, a1, c1]
                t1 = scr.tile([P, R, DIM], F32, tag="t1")
                t1v = t1[:].rearrange("p (c t) d -> p c t d", t=2)
                nc.vector.tensor_tensor(t1v, v4, dot_bc(4), MUL)  # a*v
                t2 = scrp.tile([P, R, DIM], F32, tag="t2")
                t2v = t2[:].rearrange("p (c t) d -> p c t d", t=2)
                nc.gpsimd.tensor_tensor(t2v, e4, dot_bc(0), MUL)  # b*e
                vot = io_pool.tile([P, R, DIM], F32, tag="vout")
                nc.vector.tensor_tensor(vot[:], t1[:], t2[:], ADD)

                t3 = scrp.tile([P, R, DIM], F32, tag="t3")
                t3v = t3[:].rearrange("p (c t) d -> p c t d", t=2)
                nc.gpsimd.tensor_tensor(t3v, v4, dot_bc(5), MUL)  # c*v
                t4 = scr.tile([P, R, DIM], F32, tag="t4")
                t4v = t4[:].rearrange("p (c t) d -> p c t d", t=2)
                nc.vector.tensor_tensor(t4v, e4, dot_bc(1), MUL)  # d*e
                eot = io_pool.tile([P, R, DIM], F32, tag="eout")
                nc.vector.tensor_tensor(eot[:], t3[:], t4[:], ADD)

                if with_bias:
                    bbv = bt[:, 0, :].unsqueeze(1).broadcast_to([P, R, DIM])
                    nc.vector.tensor_tensor(vot[:], vot[:], bbv, ADD)
                    bbe = bt[:, 1, :].unsqueeze(1).broadcast_to([P, R, DIM])
                    nc.vector.tensor_tensor(eot[:], eot[:], bbe, ADD)
                nc.sync.dma_start(vvo[:, sl, :], vot[:])
                nc.sync.dma_start(eeo[:, sl, :], eot[:])
    nc.compile()
    return nc


def _make_wblob(w):
    """w: [4, 64] rows = (w_vv, w_ev, w_ve, w_ee). Returns [128, 136]."""
    blob = np.zeros((P, 136), np.float32)
    # lhsT_v cols 0-3: j = (b0, d0, b1, d1); b = v.w_ev, d = v.w_ee
    blob[0:64, 0] = w[1]
    blob[0:64, 1] = w[3]
    blob[64:128, 2] = w[1]
    blob[64:128, 3] = w[3]
    # lhsT_e cols 4-7: j = (a0, c0, a1, c1); a = e.w_vv, c = e.w_ve
    blob[0:64, 4] = w[0]
    blob[0:64, 5] = w[2]
    blob[64:128, 6] = w[0]
    blob[64:128, 7] = w[2]
    blob[:, 8:136] = np.eye(P, dtype=np.float32)
    return blob


VARIANT = os.environ.get("CCU_VARIANT", "a")


def kernel(v, e, weight_vv, weight_ev, weight_ve, weight_ee, bias_v, bias_e):
    v = np.ascontiguousarray(np.asarray(v, dtype=np.float32))
    e = np.ascontiguousarray(np.asarray(e, dtype=np.float32))
    w = np.stack(
        [
            np.asarray(weight_vv, np.float32).reshape(DIM),
            np.asarray(weight_ev, np.float32).reshape(DIM),
            np.asarray(weight_ve, np.float32).reshape(DIM),
            np.asarray(weight_ee, np.float32).reshape(DIM),
        ],
        axis=0,
    )  # [4, 64]
    wrep = np.broadcast_to(w[None], (P, 4, DIM)).copy()

    bias_v = np.asarray(bias_v, np.float32).reshape(DIM)
    bias_e = np.asarray(bias_e, np.float32).reshape(DIM)
    with_bias = bool(np.any(bias_v) or np.any(bias_e))

    key = ("prog", VARIANT, with_bias)
    if key not in _cache:
        _cache[key] = (
            _build_program_c(with_bias)
            if VARIANT == "c"
            else _build_program(with_bias)
        )
    nc = _cache[key]

    in_maps = []
    for i in range(NCORES):
        m = {
            "v": v[i * BC : (i + 1) * BC],
            "e": e[i * BC : (i + 1) * BC],
        }
        if VARIANT == "c":
            m["wblob"] = _make_wblob(w)
        else:
            m["wrep"] = wrep
        if with_bias:
            brep = np.broadcast_to(
                np.stack([bias_v, bias_e], axis=0)[None], (P, 2, DIM)
            ).copy()
            m["brep"] = brep
        in_maps.append(m)

    res = run_bass_kernel_spmd(nc, in_maps, list(range(NCORES)))
    v_out = np.concatenate([res.results[i]["v_out"] for i in range(NCORES)], axis=0)
    e_out = np.concatenate([res.results[i]["e_out"] for i in range(NCORES)], axis=0)
    return (v_out, e_out)


# revision 15
# speedup vs baseline: 1.2377x; 1.2377x over previous
"""CrossCompressUnit kernel for Trainium2 (8 NeuronCores, SPMD data-parallel).

Computes, for row-batches v, e of shape (B, 64):
    a = e @ w_vv ; b = v @ w_ev ; c = e @ w_ve ; d = v @ w_ee   (per-row dots)
    v_out = v * a + e * b + bias_v
    e_out = v * c + e * d + bias_e

Sharding: pure data parallel over the batch axis across 8 cores; the tiny
(64,1) weights / (64,) biases are replicated to every core.

Layout per core (B_c = 32768 rows):
    DRAM [32768, 64] viewed as [128 partitions, 256 rows, 64]; tiles of
    R rows/partition are processed with 3D access patterns. Per-row dot
    products use a tensor_tensor multiply against a partition-replicated
    weight tile followed by a grouped (axis=X) reduce; the per-row scalars
    are then broadcast back along the dim axis with stride-0 access
    patterns.
"""

import os
import sys

import numpy as np

for _p in (
    "/root/.axon_site",
    "/root/.axon_site/_ro/trn_rl_repo",
    "/root/.axon_site/_ro/pypackages",
    "/opt/trn_rl_repo",
):
    if os.path.isdir(_p) and _p not in sys.path:
        sys.path.append(_p)

import concourse.bacc as bacc
import concourse.bass as bass
import concourse.mybir as mybir
import concourse.tile as tile
from concourse.bass_utils import run_bass_kernel_spmd

BATCH = 262144
DIM = 64
NCORES = 8
BC = BATCH // NCORES          # 32768 rows per core
P = 128                       # SBUF partitions
QROWS = BC // P               # 256 rows per partition
R = 32                        # rows per partition per tile
NT = QROWS // R               # tiles per core

F32 = mybir.dt.float32
MUL = mybir.AluOpType.mult
ADD = mybir.AluOpType.add
AX = mybir.AxisListType.X

_cache = {}


def _build_program(with_bias: bool) -> bass.Bass:
    # Bacc (not raw Bass): its compile() pass pipeline splits semaphore
    # waits (generate_event_semaphores) to satisfy the TRN2 1-wait-per-
    # instruction constraint that raw Tile output violates.
    nc = bacc.Bacc("TRN2", target_bir_lowering=False, debug=False)

    v_d = nc.dram_tensor("v", [BC, DIM], F32, kind="ExternalInput").ap()
    e_d = nc.dram_tensor("e", [BC, DIM], F32, kind="ExternalInput").ap()
    w_d = nc.dram_tensor("wrep", [P, 4, DIM], F32, kind="ExternalInput").ap()
    if with_bias:
        b_d = nc.dram_tensor("brep", [P, 2, DIM], F32, kind="ExternalInput").ap()
    vo_d = nc.dram_tensor("v_out", [BC, DIM], F32, kind="ExternalOutput").ap()
    eo_d = nc.dram_tensor("e_out", [BC, DIM], F32, kind="ExternalOutput").ap()

    vv = v_d.rearrange("(p q) d -> p q d", p=P)
    ee = e_d.rearrange("(p q) d -> p q d", p=P)
    vvo = vo_d.rearrange("(p q) d -> p q d", p=P)
    eeo = eo_d.rearrange("(p q) d -> p q d", p=P)

    with tile.TileContext(nc) as tc:
        with (
            tc.tile_pool(name="io", bufs=2) as io_pool,
            tc.tile_pool(name="scratch", bufs=2) as scratch,
            tc.tile_pool(name="dotp", bufs=2) as dotp,
            tc.tile_pool(name="const", bufs=1) as const_pool,
        ):
            # SWDGE (single completion semaphore) for the tiny constant
            # tensors: consumers already wait on the multi-queue HWDGE sems
            # of the big input DMAs, and the TT encoding has few wait slots.
            wt = const_pool.tile([P, 4, DIM], F32, tag="w")
            nc.gpsimd.dma_start(wt[:], w_d)
            if with_bias:
                bt = const_pool.tile([P, 2, DIM], F32, tag="b")
                nc.gpsimd.dma_start(bt[:], b_d)

            def wb(k):  # weight k broadcast over R rows: [P, R, DIM]
                return wt[:, k, :].unsqueeze(1).broadcast_to([P, R, DIM])

            def sb(dots_ap):  # per-row scalar broadcast along dim
                return dots_ap.unsqueeze(-1).broadcast_to([P, R, DIM])

            for j in range(NT):
                sl = slice(j * R, (j + 1) * R)
                # SWDGE loads: one completion semaphore per DMA, so the
                # first consuming TT stays within its sync-wait slot budget
                # (HWDGE sprays several queues, each with its own sem).
                vt = io_pool.tile([P, R, DIM], F32, tag="vin")
                nc.gpsimd.dma_start(vt[:], vv[:, sl, :])
                et = io_pool.tile([P, R, DIM], F32, tag="ein")
                nc.gpsimd.dma_start(et[:], ee[:, sl, :])

                dots = dotp.tile([P, 4, R], F32, tag="dots")
                # a = e.w_vv, b = v.w_ev, c = e.w_ve, d = v.w_ee
                for k, src in ((0, et), (1, vt), (2, et), (3, vt)):
                    prod = scratch.tile([P, R, DIM], F32, tag=f"prod{k}")
                    nc.vector.tensor_tensor(prod[:], src[:], wb(k), MUL)
                    nc.vector.tensor_reduce(dots[:, k, :], prod[:], axis=AX, op=ADD)

                for (ka, kb, out_ap, bias_k) in ((0, 1, vvo, 0), (2, 3, eeo, 1)):
                    t1 = scratch.tile([P, R, DIM], F32, tag="t1")
                    nc.vector.tensor_tensor(t1[:], vt[:], sb(dots[:, ka, :]), MUL)
                    t2 = scratch.tile([P, R, DIM], F32, tag="t2")
                    nc.vector.tensor_tensor(t2[:], et[:], sb(dots[:, kb, :]), MUL)
                    ot = io_pool.tile([P, R, DIM], F32, tag=f"out{bias_k}")
                    nc.vector.tensor_tensor(ot[:], t1[:], t2[:], ADD)
                    if with_bias:
                        bb = bt[:, bias_k, :].unsqueeze(1).broadcast_to([P, R, DIM])
                        nc.vector.tensor_tensor(ot[:], ot[:], bb, ADD)
                    nc.sync.dma_start(out_ap[:, sl, :], ot[:])
    nc.compile()
    return nc


def _build_program_c(with_bias: bool) -> bass.Bass:
    """PE-based dots: 2-row-packed transposes + block-diag weight matmuls.

    Per 2-row chunk c of a tile, transpose [128, 2, 64] -> [128, 128] PSUM
    (partitions = (row-parity, dim), columns = original partitions). A
    matmul with a block-diagonal [128, 4] lhsT then yields both rows' two
    dot products per column. Dots are transposed back to row-major [128, 8]
    j-layout [b0,d0,b1,d1,a0,c0,a1,c1] per chunk for the DVE combine.
    """
    nc = bacc.Bacc("TRN2", target_bir_lowering=False, debug=False)

    v_d = nc.dram_tensor("v", [BC, DIM], F32, kind="ExternalInput").ap()
    e_d = nc.dram_tensor("e", [BC, DIM], F32, kind="ExternalInput").ap()
    # wblob: [:, 0:4] lhsT_v (b,d block-diag), [:, 4:8] lhsT_e (a,c),
    # [:, 8:136] f32 identity
    w_d = nc.dram_tensor("wblob", [P, 136], F32, kind="ExternalInput").ap()
    if with_bias:
        b_d = nc.dram_tensor("brep", [P, 2, DIM], F32, kind="ExternalInput").ap()
    vo_d = nc.dram_tensor("v_out", [BC, DIM], F32, kind="ExternalOutput").ap()
    eo_d = nc.dram_tensor("e_out", [BC, DIM], F32, kind="ExternalOutput").ap()

    vv = v_d.rearrange("(p q) d -> p q d", p=P)
    ee = e_d.rearrange("(p q) d -> p q d", p=P)
    vvo = vo_d.rearrange("(p q) d -> p q d", p=P)
    eeo = eo_d.rearrange("(p q) d -> p q d", p=P)

    NCHUNK = R // 2            # 2-row chunks per tile
    GSZ = 4                    # chunks per PSUM staging group
    NGRP = NCHUNK // GSZ

    with tile.TileContext(nc) as tc:
        with (
            tc.tile_pool(name="io", bufs=2) as io_pool,
            tc.tile_pool(name="xsb", bufs=2) as xsb_pool,
            tc.tile_pool(name="dsb", bufs=2) as dsb_pool,
            tc.tile_pool(name="scrd", bufs=1) as scr,
            tc.tile_pool(name="scrp", bufs=2) as scrp,
            tc.tile_pool(name="const", bufs=1) as const_pool,
            tc.tile_pool(name="ps", bufs=2, space="PSUM") as ps_pool,
        ):
            wt = const_pool.tile([P, 136], F32, tag="w")
            nc.gpsimd.dma_start(wt[:], w_d)
            ident = wt[:, 8:136]
            if with_bias:
                bt = const_pool.tile([P, 2, DIM], F32, tag="b")
                nc.gpsimd.dma_start(bt[:], b_d)

            for j in range(NT):
                sl = slice(j * R, (j + 1) * R)
                vt = io_pool.tile([P, R, DIM], F32, tag="vin")
                nc.gpsimd.dma_start(vt[:], vv[:, sl, :])
                et = io_pool.tile([P, R, DIM], F32, tag="ein")
                nc.gpsimd.dma_start(et[:], ee[:, sl, :])

                xvsb = xsb_pool.tile([P, NCHUNK * P], F32, tag="xv")
                xesb = xsb_pool.tile([P, NCHUNK * P], F32, tag="xe")
                dots_sbv = dsb_pool.tile([4, NCHUNK * P], F32, tag="dsbv")
                dots_sbe = dsb_pool.tile([4, NCHUNK * P], F32, tag="dsbe")
                dots_t_ps = ps_pool.tile([P, NCHUNK * 8], F32, tag="dtp")

                for g in range(NGRP):
                    for (src, xsb, w_lo) in ((vt, xvsb, 0), (et, xesb, 4)):
                        xstage = ps_pool.tile([P, GSZ * P], F32, tag="xstage")
                        for i in range(GSZ):
                            c = g * GSZ + i
                            nc.tensor.transpose(
                                xstage[:, i * P : (i + 1) * P],
                                src[:, 2 * c : 2 * c + 2, :],
                                ident,
                            )
                        nc.scalar.copy(
                            xsb[:, g * GSZ * P : (g + 1) * GSZ * P], xstage[:]
                        )
                    # separate PSUM tiles: matmul out base partition must be
                    # 0/32/64 (PE quadrant rule); merge into [8, N] in SBUF
                    dots_pv = ps_pool.tile([4, GSZ * P], F32, tag="dpv")
                    dots_pe = ps_pool.tile([4, GSZ * P], F32, tag="dpe")
                    gs = slice(g * GSZ * P, (g + 1) * GSZ * P)
                    nc.tensor.matmul(
                        dots_pv[:], wt[:, 0:4], xvsb[:, gs], start=True, stop=True
                    )
                    nc.tensor.matmul(
                        dots_pe[:], wt[:, 4:8], xesb[:, gs], start=True, stop=True
                    )
                    nc.scalar.copy(dots_sbv[:, gs], dots_pv[:])
                    nc.scalar.copy(dots_sbe[:, gs], dots_pe[:])
                    for i in range(GSZ):
                        c = g * GSZ + i
                        nc.tensor.transpose(
                            dots_t_ps[:, c * 8 : c * 8 + 4],
                            dots_sbv[:, c * P : (c + 1) * P],
                            ident[0:4, 0:4],
                        )
                        nc.tensor.transpose(
                            dots_t_ps[:, c * 8 + 4 : c * 8 + 8],
                            dots_sbe[:, c * P : (c + 1) * P],
                            ident[0:4, 0:4],
                        )

                dots_t = dsb_pool.tile([P, NCHUNK * 8], F32, tag="dt")
                nc.vector.tensor_copy(dots_t[:], dots_t_ps[:])
                dview = dots_t[:].rearrange("p (c j) -> p c j", j=8)

                def dot_bc(off):
                    return (
                        dview[:, :, off : off + 3 : 2]
                        .unsqueeze(-1)
                        .broadcast_to([P, NCHUNK, 2, DIM])
                    )

                v4 = vt[:].rearrange("p (c t) d -> p c t d", t=2)
                e4 = et[:].rearrange("p (c t) d -> p c t d", t=2)

                # v_out = a*v + b*e ; e_out = c*v + d*e
                # j-layout per chunk: [b0, d0, b1, d1, a0, c0, a1, c1]
                t1 = scr.tile([P, R, DIM], F32, tag="t1")
                t1v = t1[:].rearrange("p (c t) d -> p c t d", t=2)
                nc.vector.tensor_tensor(t1v, v4, dot_bc(4), MUL)  # a*v
                t2 = scrp.tile([P, R, DIM], F32, tag="t2")
                t2v = t2[:].rearrange("p (c t) d -> p c t d", t=2)
                nc.gpsimd.tensor_tensor(t2v, e4, dot_bc(0), MUL)  # b*e
                vot = io_pool.tile([P, R, DIM], F32, tag="vout")
                nc.vector.tensor_tensor(vot[:], t1[:], t2[:], ADD)

                t3 = scrp.tile([P, R, DIM], F32, tag="t3")
                t3v = t3[:].rearrange("p (c t) d -> p c t d", t=2)
                nc.gpsimd.tensor_tensor(t3v, v4, dot_bc(5), MUL)  # c*v
                t4 = scr.tile([P, R, DIM], F32, tag="t4")
                t4v = t4[:].rearrange("p (c t) d -> p c t d", t=2)
                nc.vector.tensor_tensor(t4v, e4, dot_bc(1), MUL)  # d*e
                eot = io_pool.tile([P, R, DIM], F32, tag="eout")
                nc.vector.tensor_tensor(eot[:], t3[:], t4[:], ADD)

                if with_bias:
                    bbv = bt[:, 0, :].unsqueeze(1).broadcast_to([P, R, DIM])
                    nc.vector.tensor_tensor(vot[:], vot[:], bbv, ADD)
                    bbe = bt[:, 1, :].unsqueeze(1).broadcast_to([P, R, DIM])
                    nc.vector.tensor_tensor(eot[:], eot[:], bbe, ADD)
                nc.sync.dma_start(vvo[:, sl, :], vot[:])
                nc.sync.dma_start(eeo[:, sl, :], eot[:])
    nc.compile()
    return nc


def _make_wblob(w):
    """w: [4, 64] rows = (w_vv, w_ev, w_ve, w_ee). Returns [128, 136]."""
    blob = np.zeros((P, 136), np.float32)
    # lhsT_v cols 0-3: j = (b0, d0, b1, d1); b = v.w_ev, d = v.w_ee
    blob[0:64, 0] = w[1]
    blob[0:64, 1] = w[3]
    blob[64:128, 2] = w[1]
    blob[64:128, 3] = w[3]
    # lhsT_e cols 4-7: j = (a0, c0, a1, c1); a = e.w_vv, c = e.w_ve
    blob[0:64, 4] = w[0]
    blob[0:64, 5] = w[2]
    blob[64:128, 6] = w[0]
    blob[64:128, 7] = w[2]
    blob[:, 8:136] = np.eye(P, dtype=np.float32)
    return blob


VARIANT = os.environ.get("CCU_VARIANT", "a")


def kernel(v, e, weight_vv, weight_ev, weight_ve, weight_ee, bias_v, bias_e):
    v = np.ascontiguousarray(np.asarray(v, dtype=np.float32))
    e = np.ascontiguousarray(np.asarray(e, dtype=np.float32))
    w = np.stack(
        [
            np.asarray(weight_vv, np.float32).reshape(DIM),
            np.asarray(weight_ev, np.float32).reshape(DIM),
            np.asarray(weight_ve, np.float32).reshape(DIM),
            np.asarray(weight_ee, np.float32).reshape(DIM),
        ],
        axis=0,
    )  # [4, 64]
    wrep = np.broadcast_to(w[None], (P, 4, DIM)).copy()

    bias_v = np.asarray(bias_v, np.float32).reshape(DIM)
    bias_e = np.asarray(bias_e, np.float32).reshape(DIM)
    with_bias = bool(np.any(bias_v) or np.any(bias_e))

    key = ("prog", VARIANT, with_bias)
    if key not in _cache:
        _cache[key] = (
            _build_program_c(with_bias)
            if VARIANT == "c"
            else _build_program(with_bias)
        )
    nc = _cache[key]

    in_maps = []
    for i in range(NCORES):
        m = {
            "v": v[i * BC : (i + 1) * BC],
            "e": e[i * BC : (i + 1) * BC],
        }
        if VARIANT == "c":
            m["wblob"] = _make_wblob(w)
        else:
            m["wrep"] = wrep
        if with_bias:
            brep = np.broadcast_to(
                np.stack([bias_v, bias_e], axis=0)[None], (P, 2, DIM)
            ).copy()
            m["brep"] = brep
        in_maps.append(m)

    res = run_bass_kernel_spmd(nc, in_maps, list(range(NCORES)))
    v_out = np.concatenate([res.results[i]["v_out"] for i in range(NCORES)], axis=0)
    e_out = np.concatenate([res.results[i]["e_out"] for i in range(NCORES)], axis=0)
    return (v_out, e_out)


# revision 16
# speedup vs baseline: 847.4165x; 684.6752x over previous
"""CrossCompressUnit kernel for Trainium2 (8 NeuronCores, SPMD data-parallel).

Computes, for row-batches v, e of shape (B, 64):
    a = e @ w_vv ; b = v @ w_ev ; c = e @ w_ve ; d = v @ w_ee   (per-row dots)
    v_out = v * a + e * b + bias_v
    e_out = v * c + e * d + bias_e

Sharding: pure data parallel over the batch axis across 8 cores; the tiny
(64,1) weights / (64,) biases are replicated to every core.

Layout per core (B_c = 32768 rows):
    DRAM [32768, 64] viewed as [128 partitions, 256 rows, 64]; tiles of
    R rows/partition are processed with 3D access patterns. Per-row dot
    products use a tensor_tensor multiply against a partition-replicated
    weight tile followed by a grouped (axis=X) reduce; the per-row scalars
    are then broadcast back along the dim axis with stride-0 access
    patterns.
"""

import os
import sys

import numpy as np

for _p in (
    "/root/.axon_site",
    "/root/.axon_site/_ro/trn_rl_repo",
    "/root/.axon_site/_ro/pypackages",
    "/opt/trn_rl_repo",
):
    if os.path.isdir(_p) and _p not in sys.path:
        sys.path.append(_p)

import concourse.bacc as bacc
import concourse.bass as bass
import concourse.mybir as mybir
import concourse.tile as tile
from concourse.bass_utils import run_bass_kernel_spmd

BATCH = 262144
DIM = 64
NCORES = 8
BC = BATCH // NCORES          # 32768 rows per core
P = 128                       # SBUF partitions
QROWS = BC // P               # 256 rows per partition
R = 32                        # rows per partition per tile
NT = QROWS // R               # tiles per core

F32 = mybir.dt.float32
MUL = mybir.AluOpType.mult
ADD = mybir.AluOpType.add
AX = mybir.AxisListType.X

_cache = {}


def _build_program(with_bias: bool) -> bass.Bass:
    # Bacc (not raw Bass): its compile() pass pipeline splits semaphore
    # waits (generate_event_semaphores) to satisfy the TRN2 1-wait-per-
    # instruction constraint that raw Tile output violates.
    nc = bacc.Bacc("TRN2", target_bir_lowering=False, debug=False)

    v_d = nc.dram_tensor("v", [BC, DIM], F32, kind="ExternalInput").ap()
    e_d = nc.dram_tensor("e", [BC, DIM], F32, kind="ExternalInput").ap()
    w_d = nc.dram_tensor("wrep", [P, 4, DIM], F32, kind="ExternalInput").ap()
    if with_bias:
        b_d = nc.dram_tensor("brep", [P, 2, DIM], F32, kind="ExternalInput").ap()
    vo_d = nc.dram_tensor("v_out", [BC, DIM], F32, kind="ExternalOutput").ap()
    eo_d = nc.dram_tensor("e_out", [BC, DIM], F32, kind="ExternalOutput").ap()

    vv = v_d.rearrange("(p q) d -> p q d", p=P)
    ee = e_d.rearrange("(p q) d -> p q d", p=P)
    vvo = vo_d.rearrange("(p q) d -> p q d", p=P)
    eeo = eo_d.rearrange("(p q) d -> p q d", p=P)

    with tile.TileContext(nc) as tc:
        with (
            tc.tile_pool(name="io", bufs=2) as io_pool,
            tc.tile_pool(name="scratch", bufs=2) as scratch,
            tc.tile_pool(name="dotp", bufs=2) as dotp,
            tc.tile_pool(name="const", bufs=1) as const_pool,
        ):
            # SWDGE (single completion semaphore) for the tiny constant
            # tensors: consumers already wait on the multi-queue HWDGE sems
            # of the big input DMAs, and the TT encoding has few wait slots.
            wt = const_pool.tile([P, 4, DIM], F32, tag="w")
            nc.gpsimd.dma_start(wt[:], w_d)
            if with_bias:
                bt = const_pool.tile([P, 2, DIM], F32, tag="b")
                nc.gpsimd.dma_start(bt[:], b_d)

            def wb(k):  # weight k broadcast over R rows: [P, R, DIM]
                return wt[:, k, :].unsqueeze(1).broadcast_to([P, R, DIM])

            def sb(dots_ap):  # per-row scalar broadcast along dim
                return dots_ap.unsqueeze(-1).broadcast_to([P, R, DIM])

            for j in range(NT):
                sl = slice(j * R, (j + 1) * R)
                # SWDGE loads: one completion semaphore per DMA, so the
                # first consuming TT stays within its sync-wait slot budget
                # (HWDGE sprays several queues, each with its own sem).
                vt = io_pool.tile([P, R, DIM], F32, tag="vin")
                nc.gpsimd.dma_start(vt[:], vv[:, sl, :])
                et = io_pool.tile([P, R, DIM], F32, tag="ein")
                nc.gpsimd.dma_start(et[:], ee[:, sl, :])

                dots = dotp.tile([P, 4, R], F32, tag="dots")
                # a = e.w_vv, b = v.w_ev, c = e.w_ve, d = v.w_ee
                for k, src in ((0, et), (1, vt), (2, et), (3, vt)):
                    prod = scratch.tile([P, R, DIM], F32, tag=f"prod{k}")
                    nc.vector.tensor_tensor(prod[:], src[:], wb(k), MUL)
                    nc.vector.tensor_reduce(dots[:, k, :], prod[:], axis=AX, op=ADD)

                for (ka, kb, out_ap, bias_k) in ((0, 1, vvo, 0), (2, 3, eeo, 1)):
                    t1 = scratch.tile([P, R, DIM], F32, tag="t1")
                    nc.vector.tensor_tensor(t1[:], vt[:], sb(dots[:, ka, :]), MUL)
                    t2 = scratch.tile([P, R, DIM], F32, tag="t2")
                    nc.vector.tensor_tensor(t2[:], et[:], sb(dots[:, kb, :]), MUL)
                    ot = io_pool.tile([P, R, DIM], F32, tag=f"out{bias_k}")
                    nc.vector.tensor_tensor(ot[:], t1[:], t2[:], ADD)
                    if with_bias:
                        bb = bt[:, bias_k, :].unsqueeze(1).broadcast_to([P, R, DIM])
                        nc.vector.tensor_tensor(ot[:], ot[:], bb, ADD)
                    nc.sync.dma_start(out_ap[:, sl, :], ot[:])
    nc.compile()
    return nc


def _build_program_c(with_bias: bool) -> bass.Bass:
    """PE-based dots: 2-row-packed transposes + block-diag weight matmuls.

    Per 2-row chunk c of a tile, transpose [128, 2, 64] -> [128, 128] PSUM
    (partitions = (row-parity, dim), columns = original partitions). A
    matmul with a block-diagonal [128, 4] lhsT then yields both rows' two
    dot products per column. Dots are transposed back to row-major [128, 8]
    j-layout [b0,d0,b1,d1,a0,c0,a1,c1] per chunk for the DVE combine.
    """
    nc = bacc.Bacc("TRN2", target_bir_lowering=False, debug=False)

    v_d = nc.dram_tensor("v", [BC, DIM], F32, kind="ExternalInput").ap()
    e_d = nc.dram_tensor("e", [BC, DIM], F32, kind="ExternalInput").ap()
    # wblob: [:, 0:4] lhsT_v (b,d block-diag), [:, 4:8] lhsT_e (a,c),
    # [:, 8:136] f32 identity
    w_d = nc.dram_tensor("wblob", [P, 136], F32, kind="ExternalInput").ap()
    if with_bias:
        b_d = nc.dram_tensor("brep", [P, 2, DIM], F32, kind="ExternalInput").ap()
    vo_d = nc.dram_tensor("v_out", [BC, DIM], F32, kind="ExternalOutput").ap()
    eo_d = nc.dram_tensor("e_out", [BC, DIM], F32, kind="ExternalOutput").ap()

    vv = v_d.rearrange("(p q) d -> p q d", p=P)
    ee = e_d.rearrange("(p q) d -> p q d", p=P)
    vvo = vo_d.rearrange("(p q) d -> p q d", p=P)
    eeo = eo_d.rearrange("(p q) d -> p q d", p=P)

    NCHUNK = R // 2            # 2-row chunks per tile
    GSZ = 4                    # chunks per PSUM staging group
    NGRP = NCHUNK // GSZ

    with tile.TileContext(nc) as tc:
        with (
            tc.tile_pool(name="io", bufs=2) as io_pool,
            tc.tile_pool(name="xsb", bufs=2) as xsb_pool,
            tc.tile_pool(name="dsb", bufs=2) as dsb_pool,
            tc.tile_pool(name="scrd", bufs=1) as scr,
            tc.tile_pool(name="scrp", bufs=2) as scrp,
            tc.tile_pool(name="const", bufs=1) as const_pool,
            tc.tile_pool(name="ps", bufs=2, space="PSUM") as ps_pool,
        ):
            wt = const_pool.tile([P, 136], F32, tag="w")
            nc.gpsimd.dma_start(wt[:], w_d)
            ident = wt[:, 8:136]
            if with_bias:
                bt = const_pool.tile([P, 2, DIM], F32, tag="b")
                nc.gpsimd.dma_start(bt[:], b_d)

            for j in range(NT):
                sl = slice(j * R, (j + 1) * R)
                vt = io_pool.tile([P, R, DIM], F32, tag="vin")
                nc.gpsimd.dma_start(vt[:], vv[:, sl, :])
                et = io_pool.tile([P, R, DIM], F32, tag="ein")
                nc.gpsimd.dma_start(et[:], ee[:, sl, :])

                xvsb = xsb_pool.tile([P, NCHUNK * P], F32, tag="xv")
                xesb = xsb_pool.tile([P, NCHUNK * P], F32, tag="xe")
                dots_sbv = dsb_pool.tile([4, NCHUNK * P], F32, tag="dsbv")
                dots_sbe = dsb_pool.tile([4, NCHUNK * P], F32, tag="dsbe")
                dots_t_ps = ps_pool.tile([P, NCHUNK * 8], F32, tag="dtp")

                for g in range(NGRP):
                    for (src, xsb, w_lo) in ((vt, xvsb, 0), (et, xesb, 4)):
                        xstage = ps_pool.tile([P, GSZ * P], F32, tag="xstage")
                        for i in range(GSZ):
                            c = g * GSZ + i
                            nc.tensor.transpose(
                                xstage[:, i * P : (i + 1) * P],
                                src[:, 2 * c : 2 * c + 2, :],
                                ident,
                            )
                        nc.scalar.copy(
                            xsb[:, g * GSZ * P : (g + 1) * GSZ * P], xstage[:]
                        )
                    # separate PSUM tiles: matmul out base partition must be
                    # 0/32/64 (PE quadrant rule); merge into [8, N] in SBUF
                    dots_pv = ps_pool.tile([4, GSZ * P], F32, tag="dpv")
                    dots_pe = ps_pool.tile([4, GSZ * P], F32, tag="dpe")
                    gs = slice(g * GSZ * P, (g + 1) * GSZ * P)
                    nc.tensor.matmul(
                        dots_pv[:], wt[:, 0:4], xvsb[:, gs], start=True, stop=True
                    )
                    nc.tensor.matmul(
                        dots_pe[:], wt[:, 4:8], xesb[:, gs], start=True, stop=True
                    )
                    nc.scalar.copy(dots_sbv[:, gs], dots_pv[:])
                    nc.scalar.copy(dots_sbe[:, gs], dots_pe[:])
                    for i in range(GSZ):
                        c = g * GSZ + i
                        nc.tensor.transpose(
                            dots_t_ps[:, c * 8 : c * 8 + 4],
                            dots_sbv[:, c * P : (c + 1) * P],
                            ident[0:4, 0:4],
                        )
                        nc.tensor.transpose(
                            dots_t_ps[:, c * 8 + 4 : c * 8 + 8],
                            dots_sbe[:, c * P : (c + 1) * P],
                            ident[0:4, 0:4],
                        )

                dots_t = dsb_pool.tile([P, NCHUNK * 8], F32, tag="dt")
                nc.vector.tensor_copy(dots_t[:], dots_t_ps[:])
                dview = dots_t[:].rearrange("p (c j) -> p c j", j=8)

                def dot_bc(off):
                    return (
                        dview[:, :, off : off + 3 : 2]
                        .unsqueeze(-1)
                        .broadcast_to([P, NCHUNK, 2, DIM])
                    )

                v4 = vt[:].rearrange("p (c t) d -> p c t d", t=2)
                e4 = et[:].rearrange("p (c t) d -> p c t d", t=2)

                # v_out = a*v + b*e ; e_out = c*v + d*e
                # j-layout per chunk: [b0, d0, b1, d1, a0, c0, a1, c1]
                t1 = scr.tile([P, R, DIM], F32, tag="t1")
                t1v = t1[:].rearrange("p (c t) d -> p c t d", t=2)
                nc.vector.tensor_tensor(t1v, v4, dot_bc(4), MUL)  # a*v
                t2 = scrp.tile([P, R, DIM], F32, tag="t2")
                t2v = t2[:].rearrange("p (c t) d -> p c t d", t=2)
                nc.gpsimd.tensor_tensor(t2v, e4, dot_bc(0), MUL)  # b*e
                vot = io_pool.tile([P, R, DIM], F32, tag="vout")
                nc.vector.tensor_tensor(vot[:], t1[:], t2[:], ADD)

                t3 = scrp.tile([P, R, DIM], F32, tag="t3")
                t3v = t3[:].rearrange("p (c t) d -> p c t d", t=2)
                nc.gpsimd.tensor_tensor(t3v, v4, dot_bc(5), MUL)  # c*v
                t4 = scr.tile([P, R, DIM], F32, tag="t4")
                t4v = t4[:].rearrange("p (c t) d -> p c t d", t=2)
                nc.vector.tensor_tensor(t4v, e4, dot_bc(1), MUL)  # d*e
                eot = io_pool.tile([P, R, DIM], F32, tag="eout")
                nc.vector.tensor_tensor(eot[:], t3[:], t4[:], ADD)

                if with_bias:
                    bbv = bt[:, 0, :].unsqueeze(1).broadcast_to([P, R, DIM])
                    nc.vector.tensor_tensor(vot[:], vot[:], bbv, ADD)
                    bbe = bt[:, 1, :].unsqueeze(1).broadcast_to([P, R, DIM])
                    nc.vector.tensor_tensor(eot[:], eot[:], bbe, ADD)
                nc.sync.dma_start(vvo[:, sl, :], vot[:])
                nc.sync.dma_start(eeo[:, sl, :], eot[:])
    nc.compile()
    return nc


def _make_wblob(w):
    """w: [4, 64] rows = (w_vv, w_ev, w_ve, w_ee). Returns [128, 136]."""
    blob = np.zeros((P, 136), np.float32)
    # lhsT_v cols 0-3: j = (b0, d0, b1, d1); b = v.w_ev, d = v.w_ee
    blob[0:64, 0] = w[1]
    blob[0:64, 1] = w[3]
    blob[64:128, 2] = w[1]
    blob[64:128, 3] = w[3]
    # lhsT_e cols 4-7: j = (a0, c0, a1, c1); a = e.w_vv, c = e.w_ve
    blob[0:64, 4] = w[0]
    blob[0:64, 5] = w[2]
    blob[64:128, 6] = w[0]
    blob[64:128, 7] = w[2]
    blob[:, 8:136] = np.eye(P, dtype=np.float32)
    return blob


# "c" (default): PE-transpose dot products + DVE/GPSIMD combine — cost
# model ~110us/core, DMA-bound. "a": all-DVE fallback (~240us/core).
VARIANT = os.environ.get("CCU_VARIANT", "c")


def kernel(v, e, weight_vv, weight_ev, weight_ve, weight_ee, bias_v, bias_e):
    v = np.ascontiguousarray(np.asarray(v, dtype=np.float32))
    e = np.ascontiguousarray(np.asarray(e, dtype=np.float32))
    w = np.stack(
        [
            np.asarray(weight_vv, np.float32).reshape(DIM),
            np.asarray(weight_ev, np.float32).reshape(DIM),
            np.asarray(weight_ve, np.float32).reshape(DIM),
            np.asarray(weight_ee, np.float32).reshape(DIM),
        ],
        axis=0,
    )  # [4, 64]
    wrep = np.broadcast_to(w[None], (P, 4, DIM)).copy()

    bias_v = np.asarray(bias_v, np.float32).reshape(DIM)
    bias_e = np.asarray(bias_e, np.float32).reshape(DIM)
    with_bias = bool(np.any(bias_v) or np.any(bias_e))

    key = ("prog", VARIANT, with_bias)
    if key not in _cache:
        _cache[key] = (
            _build_program_c(with_bias)
            if VARIANT == "c"
            else _build_program(with_bias)
        )
    nc = _cache[key]

    in_maps = []
    for i in range(NCORES):
        m = {
            "v": v[i * BC : (i + 1) * BC],
            "e": e[i * BC : (i + 1) * BC],
        }
        if VARIANT == "c":
            m["wblob"] = _make_wblob(w)
        else:
            m["wrep"] = wrep
        if with_bias:
            brep = np.broadcast_to(
                np.stack([bias_v, bias_e], axis=0)[None], (P, 2, DIM)
            ).copy()
            m["brep"] = brep
        in_maps.append(m)

    res = run_bass_kernel_spmd(nc, in_maps, list(range(NCORES)))
    v_out = np.concatenate([res.results[i]["v_out"] for i in range(NCORES)], axis=0)
    e_out = np.concatenate([res.results[i]["e_out"] for i in range(NCORES)], axis=0)
    return (v_out, e_out)
